# revision 6
# baseline (speedup 1.0000x reference)
"""Brownian/OU bridge sampler kernel for Trainium2 (8 NeuronCores).

Problem (per batch element b, time series of length T, DIM=64 channels):
  first 32 channels:  bm = cumsum_t(sqrt(dt)*noise) / (sqrt(t)+1e-8)
  last 32 channels:   ou = e^{-theta t} * cumsum_t(sqrt(e^{2 theta t}-e^{2 theta t'})
                           * sigma/sqrt(2 theta) * noise)
                           / (sigma*sqrt((1-e^{-2 theta t})/(2 theta))+1e-8)

Strategy: pure data parallel over batch (32 samples per core); no cross-core
communication. Each 256-timestep chunk is loaded with a time-PAIR layout —
partition p holds t = 2p and 2p+1 — so every DMA descriptor covers 512 B
(the SDMA line-rate knee; a plain t-per-partition layout yields 256 B
descriptors and only ~134 GB/s vs ~300 GB/s measured for this pattern).
The 256-step cumsum is built from fp32 matmuls against a triangular-ones
stationary: S_odd = L^T r_even + L^T r_odd (+ carry), then
S_even = S_odd - r_odd on the vector engine. Cross-chunk carries ride a
third matmul whose stationary selects PSUM row 127 (the running total) out
of an aligned 32-row stash copied by the scalar engine. Per-timestep
coefficients are precomputed once on a compact [128, 512] layout (flat
full-speed ts load + PE-transpose redistribution) and broadcast into the
bulk passes with step-0 access patterns.

Numerics: exp(2θt)-exp(2θt') is restructured as exp(2θt')*expm1(2θ dt)
(Taylor expm1; dt<=1e-2 so a cubic is exact to 3e-10) and 1-exp(-2θt) uses
a degree-6 Taylor/direct blend at 2θt=0.5 — both avoid catastrophic
cancellation against the ACT engine's ~1e-5 exp error. Everything else is
fp32; the result matches a float64 pipeline to 1.15e-4, which is the
reference's own fp32 noise floor.
"""
import numpy as np

import bass_rust
import concourse.bass as bass
import concourse.tile as tile
from concourse import mybir
from concourse.bass_utils import run_bass_kernel_spmd

B, T, DIM = 256, 2048, 64
THETA = 0.1
N_CORES = 8
NB = B // N_CORES      # 32 samples per core
P = 128                # partitions
NC2 = T // (2 * P)     # 8 time chunks of 256 steps
S = 8                  # samples packed per matmul free dim
G = NB // S            # 4 carry chains per core
H = 2                  # halves (bm / ou)
DH = DIM // H          # 32
FREE = S * DIM         # 512 = one PSUM bank of fp32
L2 = 2                 # time-pair dimension

F32 = mybir.dt.float32
AF = mybir.ActivationFunctionType
OP = mybir.AluOpType


def _split_waits(nc, max_waits=1):
    """walrus in this container rejects >1 sem wait per instruction; hoist
    extras onto same-engine NoOps inserted just before the offender."""
    n = 0
    for f in nc.m.functions:
        for blk in f.blocks:
            insts = blk.instructions
            i = 0
            while i < len(insts):
                inst = insts[i]
                si = inst.sync_info
                if si is not None and len(si.on_wait) > max_waits:
                    waits = list(si.on_wait)
                    keep, rest = waits[:max_waits], waits[max_waits:]
                    nops = []
                    for j in range(0, len(rest), max_waits):
                        nop = bass_rust.InstNoOp(name=f"I-ws-{n}", ins=[], outs=[])
                        n += 1
                        nop.engine = inst.engine
                        nop.sync_info = mybir.SyncInfo(
                            on_wait=rest[j : j + max_waits], on_update=[])
                        nops.append(nop)
                    inst.sync_info = mybir.SyncInfo(
                        on_wait=keep, on_update=list(si.on_update))
                    for k, nop in enumerate(nops):
                        insts.insert(i + k, nop)
                    i += len(nops)
                i += 1
    return nc


def _strided(ap_full, offset_elems, step, count):
    """[P, count] view of a tile's free space at element offset with stride."""
    return bass.AP(
        tensor=ap_full.tensor,
        offset=ap_full.offset + offset_elems,
        ap=[list(ap_full.ap[0]), [step, count]],
    )


def _build(reps: int = 1, hw_loop: int = 0):
    nc = bass.Bass("TRN2")
    ts_in = nc.dram_tensor("ts", [NB, T, 1], F32, kind="ExternalInput")
    nz_in = nc.dram_tensor("noise", [NB, T, DIM], F32, kind="ExternalInput")
    out = nc.dram_tensor("out", [NB, T, DIM], F32, kind="ExternalOutput")

    ts_flat = ts_in[:, :, 0].rearrange("s t -> (s t)")

    with tile.TileContext(nc) as tc:
        with (
            tc.tile_pool(name="consts", bufs=1) as consts,
            tc.tile_pool(name="cwork", bufs=1) as cwork,
            tc.tile_pool(name="nzp", bufs=3) as nzp,
            tc.tile_pool(name="rp", bufs=6) as rp,
            tc.tile_pool(name="tep", bufs=6) as tep,
            tc.tile_pool(name="op_", bufs=3) as op_,
            tc.tile_pool(name="psp", bufs=6, space="PSUM") as psp,
        ):
            # ---------------- constants ----------------
            ones_t = consts.tile([P, P], F32)
            nc.vector.memset(ones_t, 1.0)
            L = consts.tile([P, P], F32)          # L[u, q] = 1 if u <= q
            nc.gpsimd.affine_select(
                out=L, in_=ones_t, pattern=[[1, P]], compare_op=OP.is_ge,
                fill=0.0, base=0, channel_multiplier=-1)
            e31 = consts.tile([32, P], F32)       # row 31 ones, else 0
            nc.gpsimd.affine_select(
                out=e31, in_=ones_t[0:32, :], pattern=[[0, P]],
                compare_op=OP.is_equal, fill=0.0, base=-31,
                channel_multiplier=1)
            ident = consts.tile([P, P], F32)      # identity for PE transpose
            nc.gpsimd.affine_select(
                out=ident, in_=ones_t, pattern=[[-1, P]],
                compare_op=OP.is_equal, fill=0.0, base=0,
                channel_multiplier=1)

            # -------- compact per-timestep coefficients --------
            # Target layout: X[p, g, s', i2, l] holds t = i2*256 + 2p + l of
            # sample s = 8g + s'; flat free index = 2n + l with n = s*8 + i2.
            # Filled from a flat full-speed ts load via PE transposes with
            # stride-2 input APs: T_{hb,l}[p, q] = flat[q*512 + 256*hb + 2p + l]
            # lands at n = 2q + hb (free stride 4, offset 2*hb + l).
            s1 = consts.tile([P, 512], F32)       # flat[p*512 + f]
            nc.sync.dma_start(
                out=s1, in_=ts_flat.rearrange("(p f) -> p f", p=P))
            s1p = consts.tile([P, 512], F32)      # flat[p*512 + f - 1]
            nc.sync.dma_start(
                out=s1p[1:P, :],
                in_=bass.AP(tensor=ts_flat.tensor, offset=ts_flat.offset + 511,
                            ap=[[512, P - 1], [1, 512]]))
            nc.sync.dma_start(
                out=s1p[0:1, 1:512],
                in_=bass.AP(tensor=ts_flat.tensor, offset=ts_flat.offset,
                            ap=[[0, 1], [1, 511]]))
            nc.vector.memset(s1p[0:1, 0:1], 0.0)

            ts_c = consts.tile([P, G, S, NC2, L2], F32)
            tsp_c = consts.tile([P, G, S, NC2, L2], F32)
            tsf = ts_c[:, :, :, :, :].rearrange("p g s i l -> p (g s i l)")
            tspf = tsp_c[:, :, :, :, :].rearrange("p g s i l -> p (g s i l)")
            with tc.tile_pool(name="trps", bufs=2, space="PSUM") as trps:
                for src, dstf in ((s1, tsf), (s1p, tspf)):
                    for hb in range(2):
                        for lv in range(2):
                            pst = trps.tile([P, P], F32, tag="trp",
                                            name=f"trp{hb}{lv}")
                            nc.tensor.transpose(
                                out=pst,
                                in_=_strided(src[:, :], 256 * hb + lv, 2, P),
                                identity=ident)
                            nc.vector.tensor_copy(
                                out=_strided(dstf, 2 * hb + lv, 4, P),
                                in_=pst)
            # each sample's t=0 has predecessor time 0
            nc.vector.memset(tsp_c[0:1, :, :, 0:1, 0:1], 0.0)

            cmul = consts.tile([P, G, S, NC2, L2, H], F32)
            cnorm = consts.tile([P, G, S, NC2, L2, H], F32)
            cm0 = cmul[:, :, :, :, :, 0].rearrange("p g s i l -> p (g s i l)")
            cm1 = cmul[:, :, :, :, :, 1].rearrange("p g s i l -> p (g s i l)")
            cn0 = cnorm[:, :, :, :, :, 0].rearrange("p g s i l -> p (g s i l)")
            cn1 = cnorm[:, :, :, :, :, 1].rearrange("p g s i l -> p (g s i l)")

            NF = G * S * NC2 * L2  # 512
            t0 = cwork.tile([P, NF], F32, tag="t0")
            t1 = cwork.tile([P, NF], F32, tag="t1")
            t2 = cwork.tile([P, NF], F32, tag="t2")
            t3 = cwork.tile([P, NF], F32, tag="t3")
            t4 = cwork.tile([P, NF], F32, tag="t4")

            # db = sqrt(ts - tsp)   (fp32 subtraction is exact here)
            nc.vector.tensor_tensor(out=t0, in0=tsf, in1=tspf, op=OP.subtract)
            nc.scalar.activation(out=cm0, in_=t0, func=AF.Sqrt)
            # dou = sqrt(5 * exp(.2 tsp) * expm1(.2 (ts-tsp)))
            nc.vector.tensor_scalar_mul(out=t1, in0=t0, scalar1=0.2)     # x
            nc.vector.tensor_scalar(out=t2, in0=t1, scalar1=1.0 / 3.0,
                                    scalar2=1.0, op0=OP.mult, op1=OP.add)
            nc.vector.tensor_mul(out=t3, in0=t1, in1=t2)
            nc.vector.tensor_scalar(out=t2, in0=t3, scalar1=0.5,
                                    scalar2=1.0, op0=OP.mult, op1=OP.add)
            nc.vector.tensor_mul(out=t3, in0=t1, in1=t2)                 # expm1
            nc.scalar.activation(out=t2, in_=tspf, func=AF.Exp, scale=0.2)
            nc.vector.tensor_mul(out=t3, in0=t3, in1=t2)
            nc.scalar.activation(out=cm1, in_=t3, func=AF.Sqrt, scale=5.0)
            # nb = 1/(sqrt(ts)+1e-8)
            nc.scalar.activation(out=t0, in_=tsf, func=AF.Sqrt)
            nc.vector.tensor_scalar_add(out=t0, in0=t0, scalar1=1e-8)
            nc.vector.reciprocal(out=cn0, in_=t0)
            # f2 = exp(-.1 ts) / (sqrt(5*(1-exp(-.2 ts))) + 1e-8)
            #   1-exp(-y), y = .2 ts: Taylor (deg 6) below y=0.5 else direct
            nc.vector.tensor_scalar_mul(out=t0, in0=tsf, scalar1=0.2)    # y
            nc.scalar.activation(out=t1, in_=tsf, func=AF.Exp, scale=-0.2)
            nc.vector.tensor_scalar(out=t1, in0=t1, scalar1=-1.0,
                                    scalar2=1.0, op0=OP.mult, op1=OP.add)
            nc.vector.tensor_scalar(out=t2, in0=t0, scalar1=-1.0 / 6.0,
                                    scalar2=1.0, op0=OP.mult, op1=OP.add)
            for k in (5, 4, 3, 2):
                nc.vector.tensor_mul(out=t3, in0=t0, in1=t2)
                nc.vector.tensor_scalar(out=t2, in0=t3, scalar1=-1.0 / k,
                                        scalar2=1.0, op0=OP.mult, op1=OP.add)
            nc.vector.tensor_mul(out=t3, in0=t0, in1=t2)                 # taylor
            nc.vector.tensor_scalar(out=t4, in0=t0, scalar1=0.5, scalar2=None,
                                    op0=OP.is_lt)
            nc.vector.tensor_tensor(out=t3, in0=t3, in1=t1, op=OP.subtract)
            nc.vector.tensor_mul(out=t3, in0=t4, in1=t3)
            nc.vector.tensor_tensor(out=t3, in0=t3, in1=t1, op=OP.add)   # w2
            nc.scalar.activation(out=t3, in_=t3, func=AF.Sqrt, scale=5.0)
            nc.vector.tensor_scalar_add(out=t3, in0=t3, scalar1=1e-8)
            nc.vector.reciprocal(out=t3, in_=t3)
            nc.scalar.activation(out=t0, in_=tsf, func=AF.Exp, scale=-0.1)
            nc.vector.tensor_mul(out=cn1, in0=t0, in1=t3)

            # ---------------- main scan ----------------
            ctmp = [consts.tile([32, FREE], F32, tag=f"ctmp{g}", name=f"ctmp{g}")
                    for g in range(G)]
            # float32r views: same fp32 bits, but the PE runs the matmul at
            # 1 cycle/row instead of fp32's 4 (output free size 512 >= 256).
            F32R = mybir.dt.float32r
            LR = L[:, :].bitcast(F32R)
            E31R = e31[:, :].bitcast(F32R)

            def emit_reps():
              for _rep in range(reps):
                for i2 in range(NC2):
                    tsl = slice(i2 * 2 * P, (i2 + 1) * 2 * P)
                    # one 2 MiB load per 256-step chunk (all 32 samples) on
                    # the SP HWDGE ring; stores go on the ACT ring below.
                    nt = nzp.tile([P, NB, L2 * DIM], F32, tag="nz")
                    nc.sync.dma_start(
                        out=nt,
                        in_=nz_in[:, tsl, :].rearrange(
                            "s (p l) d -> p s (l d)", l=L2))
                    o = op_.tile([P, NB, L2 * DIM], F32, tag="o")
                    for g in range(G):
                        sgs = slice(g * S, (g + 1) * S)
                        r = rp.tile([P, S, L2, DIM], F32, tag="r")
                        nc.vector.tensor_mul(
                            out=r[:, :, :, :].rearrange(
                                "p s l (h d) -> p s l h d", h=H),
                            in0=nt[:, sgs, :].rearrange(
                                "p s (l h d) -> p s l h d", l=L2, h=H),
                            in1=cmul[:, g, :, i2, :, :].to_broadcast(
                                [P, S, L2, H, DH]))
                        ps = psp.tile([P, FREE], F32, tag="ps")
                        re = r[:, :, 0, :]
                        ro = r[:, :, 1, :]
                        nc.tensor.matmul(ps, lhsT=LR, rhs=re.bitcast(F32R),
                                         start=True, stop=False)
                        nc.tensor.matmul(ps, lhsT=LR, rhs=ro.bitcast(F32R),
                                         start=False, stop=(i2 == 0))
                        if i2 > 0:
                            nc.tensor.matmul(ps, lhsT=E31R,
                                             rhs=ctmp[g][:, :].bitcast(F32R),
                                             start=False, stop=True)
                        if i2 < NC2 - 1:
                            nc.scalar.activation(out=ctmp[g], in_=ps[96:128, :],
                                                 func=AF.Copy)
                        # odd time positions: S_odd * cnorm
                        nc.vector.tensor_mul(
                            out=o[:, sgs, DIM : L2 * DIM].rearrange(
                                "p s (h d) -> p s h d", h=H),
                            in0=ps[:, :].rearrange(
                                "p (s h d) -> p s h d", s=S, h=H),
                            in1=cnorm[:, g, :, i2, 1, :].to_broadcast(
                                [P, S, H, DH]))
                        # even time positions: (S_odd - r_odd) * cnorm
                        te = tep.tile([P, S, DIM], F32, tag="te")
                        nc.gpsimd.tensor_tensor(
                            out=te,
                            in0=ps[:, :].rearrange("p (s d) -> p s d", s=S),
                            in1=ro, op=OP.subtract)
                        mul_eng = nc.vector if g % 2 == 0 else nc.gpsimd
                        mul_eng.tensor_mul(
                            out=o[:, sgs, 0:DIM].rearrange(
                                "p s (h d) -> p s h d", h=H),
                            in0=te[:, :, :].rearrange(
                                "p s (h d) -> p s h d", h=H),
                            in1=cnorm[:, g, :, i2, 0, :].to_broadcast(
                                [P, S, H, DH]))
                    nc.scalar.dma_start(
                        out=out[:, tsl, :].rearrange(
                            "s (p l) d -> p s (l d)", l=L2),
                        in_=o)

            if hw_loop:
                with tc.For_i(0, hw_loop):
                    emit_reps()
            else:
                emit_reps()
    _split_waits(nc)
    return nc


_NC = None


def _get_nc():
    global _NC
    if _NC is None:
        _NC = _build()
    return _NC


def kernel(ts: np.ndarray, noise: np.ndarray) -> np.ndarray:
    ts = np.ascontiguousarray(ts, dtype=np.float32)
    noise = np.ascontiguousarray(noise, dtype=np.float32)
    in_maps = [
        {"ts": ts[c * NB : (c + 1) * NB], "noise": noise[c * NB : (c + 1) * NB]}
        for c in range(N_CORES)
    ]
    res = run_bass_kernel_spmd(_get_nc(), in_maps, core_ids=list(range(N_CORES)))
    return np.concatenate([r["out"] for r in res.results], axis=0)



# revision 10
# speedup vs baseline: 1.3974x; 1.3974x over previous
"""Brownian/OU bridge sampler kernel for Trainium2 (8 NeuronCores).

Problem (per batch element b, time series of length T, DIM=64 channels):
  first 32 channels:  bm = cumsum_t(sqrt(dt)*noise) / (sqrt(t)+1e-8)
  last 32 channels:   ou = e^{-theta t} * cumsum_t(sqrt(e^{2 theta t}-e^{2 theta t'})
                           * sigma/sqrt(2 theta) * noise)
                           / (sigma*sqrt((1-e^{-2 theta t})/(2 theta))+1e-8)

Strategy: pure data parallel over batch (32 samples per core); no cross-core
communication. Each 256-timestep chunk is loaded with a time-PAIR layout —
partition p holds t = 2p and 2p+1 — so every DMA descriptor covers 512 B
(the SDMA line-rate knee; a plain t-per-partition layout yields 256 B
descriptors and only ~134 GB/s vs ~300 GB/s measured for this pattern).
The 256-step cumsum is built from fp32 matmuls against a triangular-ones
stationary: S_odd = L^T r_even + L^T r_odd (+ carry), then
S_even = S_odd - r_odd on the vector engine. Cross-chunk carries ride a
third matmul whose stationary selects PSUM row 127 (the running total) out
of an aligned 32-row stash copied by the scalar engine. Per-timestep
coefficients are precomputed once on a compact [128, 512] layout (flat
full-speed ts load + PE-transpose redistribution) and broadcast into the
bulk passes with step-0 access patterns.

Numerics: exp(2θt)-exp(2θt') is restructured as exp(2θt')*expm1(2θ dt)
(Taylor expm1; dt<=1e-2 so a cubic is exact to 3e-10) and 1-exp(-2θt) uses
a degree-6 Taylor/direct blend at 2θt=0.5 — both avoid catastrophic
cancellation against the ACT engine's ~1e-5 exp error. Everything else is
fp32; the result matches a float64 pipeline to 1.15e-4, which is the
reference's own fp32 noise floor.
"""
import numpy as np

import bass_rust
import concourse.bass as bass
import concourse.tile as tile
from concourse import mybir
from concourse.bass_utils import run_bass_kernel_spmd

B, T, DIM = 256, 2048, 64
THETA = 0.1
N_CORES = 8
NB = B // N_CORES      # 32 samples per core
P = 128                # partitions
NC2 = T // (2 * P)     # 8 time chunks of 256 steps
S = 8                  # samples packed per matmul free dim
G = NB // S            # 4 carry chains per core
H = 2                  # halves (bm / ou)
DH = DIM // H          # 32
FREE = S * DIM         # 512 = one PSUM bank of fp32
L2 = 2                 # time-pair dimension

F32 = mybir.dt.float32
AF = mybir.ActivationFunctionType
OP = mybir.AluOpType


def _split_waits(nc, max_waits=1):
    """walrus in this container rejects >1 sem wait per instruction; hoist
    extras onto same-engine NoOps inserted just before the offender."""
    n = 0
    for f in nc.m.functions:
        for blk in f.blocks:
            insts = blk.instructions
            i = 0
            while i < len(insts):
                inst = insts[i]
                si = inst.sync_info
                if si is not None and len(si.on_wait) > max_waits:
                    waits = list(si.on_wait)
                    keep, rest = waits[:max_waits], waits[max_waits:]
                    nops = []
                    for j in range(0, len(rest), max_waits):
                        nop = bass_rust.InstNoOp(name=f"I-ws-{n}", ins=[], outs=[])
                        n += 1
                        nop.engine = inst.engine
                        nop.sync_info = mybir.SyncInfo(
                            on_wait=rest[j : j + max_waits], on_update=[])
                        nops.append(nop)
                    inst.sync_info = mybir.SyncInfo(
                        on_wait=keep, on_update=list(si.on_update))
                    for k, nop in enumerate(nops):
                        insts.insert(i + k, nop)
                    i += len(nops)
                i += 1
    return nc


def _strided(ap_full, offset_elems, step, count):
    """[P, count] view of a tile's free space at element offset with stride."""
    return bass.AP(
        tensor=ap_full.tensor,
        offset=ap_full.offset + offset_elems,
        ap=[list(ap_full.ap[0]), [step, count]],
    )


def _build(reps: int = 1, hw_loop: int = 0):
    nc = bass.Bass("TRN2")
    ts_in = nc.dram_tensor("ts", [NB, T, 1], F32, kind="ExternalInput")
    nz_in = nc.dram_tensor("noise", [NB, T, DIM], F32, kind="ExternalInput")
    out = nc.dram_tensor("out", [NB, T, DIM], F32, kind="ExternalOutput")

    ts_flat = ts_in[:, :, 0].rearrange("s t -> (s t)")

    with tile.TileContext(nc) as tc:
        with (
            tc.tile_pool(name="consts", bufs=1) as consts,
            tc.tile_pool(name="cwork", bufs=1) as cwork,
            tc.tile_pool(name="nzp", bufs=3) as nzp,
            tc.tile_pool(name="rp", bufs=6) as rp,
            tc.tile_pool(name="tep", bufs=6) as tep,
            tc.tile_pool(name="op_", bufs=3) as op_,
            tc.tile_pool(name="psp", bufs=6, space="PSUM") as psp,
        ):
            # ---------------- constants ----------------
            ones_t = consts.tile([P, P], F32)
            nc.vector.memset(ones_t, 1.0)
            L = consts.tile([P, P], F32)          # L[u, q] = 1 if u <= q
            nc.gpsimd.affine_select(
                out=L, in_=ones_t, pattern=[[1, P]], compare_op=OP.is_ge,
                fill=0.0, base=0, channel_multiplier=-1)
            e31 = consts.tile([32, P], F32)       # row 31 ones, else 0
            nc.gpsimd.affine_select(
                out=e31, in_=ones_t[0:32, :], pattern=[[0, P]],
                compare_op=OP.is_equal, fill=0.0, base=-31,
                channel_multiplier=1)
            ident = consts.tile([P, P], F32)      # identity for PE transpose
            nc.gpsimd.affine_select(
                out=ident, in_=ones_t, pattern=[[-1, P]],
                compare_op=OP.is_equal, fill=0.0, base=0,
                channel_multiplier=1)

            # -------- compact per-timestep coefficients --------
            # Target layout: X[p, g, s', i2, l] holds t = i2*256 + 2p + l of
            # sample s = 8g + s'; flat free index = 2n + l with n = s*8 + i2.
            # Filled from a flat full-speed ts load via PE transposes with
            # stride-2 input APs: T_{hb,l}[p, q] = flat[q*512 + 256*hb + 2p + l]
            # lands at n = 2q + hb (free stride 4, offset 2*hb + l).
            s1 = consts.tile([P, 512], F32)       # flat[p*512 + f]
            nc.sync.dma_start(
                out=s1, in_=ts_flat.rearrange("(p f) -> p f", p=P))
            s1p = consts.tile([P, 512], F32)      # flat[p*512 + f - 1]
            nc.sync.dma_start(
                out=s1p[1:P, :],
                in_=bass.AP(tensor=ts_flat.tensor, offset=ts_flat.offset + 511,
                            ap=[[512, P - 1], [1, 512]]))
            nc.sync.dma_start(
                out=s1p[0:1, 1:512],
                in_=bass.AP(tensor=ts_flat.tensor, offset=ts_flat.offset,
                            ap=[[0, 1], [1, 511]]))
            nc.vector.memset(s1p[0:1, 0:1], 0.0)

            ts_c = consts.tile([P, G, S, NC2, L2], F32)
            tsp_c = consts.tile([P, G, S, NC2, L2], F32)
            tsf = ts_c[:, :, :, :, :].rearrange("p g s i l -> p (g s i l)")
            tspf = tsp_c[:, :, :, :, :].rearrange("p g s i l -> p (g s i l)")
            with tc.tile_pool(name="trps", bufs=2, space="PSUM") as trps:
                for src, dstf in ((s1, tsf), (s1p, tspf)):
                    for hb in range(2):
                        for lv in range(2):
                            pst = trps.tile([P, P], F32, tag="trp",
                                            name=f"trp{hb}{lv}")
                            nc.tensor.transpose(
                                out=pst,
                                in_=_strided(src[:, :], 256 * hb + lv, 2, P),
                                identity=ident)
                            nc.vector.tensor_copy(
                                out=_strided(dstf, 2 * hb + lv, 4, P),
                                in_=pst)
            # each sample's t=0 has predecessor time 0
            nc.vector.memset(tsp_c[0:1, :, :, 0:1, 0:1], 0.0)

            cmul = consts.tile([P, G, S, NC2, L2, H], F32)
            cnorm = consts.tile([P, G, S, NC2, L2, H], F32)
            cm0 = cmul[:, :, :, :, :, 0].rearrange("p g s i l -> p (g s i l)")
            cm1 = cmul[:, :, :, :, :, 1].rearrange("p g s i l -> p (g s i l)")
            cn0 = cnorm[:, :, :, :, :, 0].rearrange("p g s i l -> p (g s i l)")
            cn1 = cnorm[:, :, :, :, :, 1].rearrange("p g s i l -> p (g s i l)")

            NF = G * S * NC2 * L2  # 512
            t0 = cwork.tile([P, NF], F32, tag="t0")
            t1 = cwork.tile([P, NF], F32, tag="t1")
            t2 = cwork.tile([P, NF], F32, tag="t2")
            t3 = cwork.tile([P, NF], F32, tag="t3")
            t4 = cwork.tile([P, NF], F32, tag="t4")

            # db = sqrt(ts - tsp)   (fp32 subtraction is exact here)
            nc.vector.tensor_tensor(out=t0, in0=tsf, in1=tspf, op=OP.subtract)
            nc.scalar.activation(out=cm0, in_=t0, func=AF.Sqrt)
            # dou = sqrt(5 * exp(.2 tsp) * expm1(.2 (ts-tsp)))
            nc.vector.tensor_scalar_mul(out=t1, in0=t0, scalar1=0.2)     # x
            nc.vector.tensor_scalar(out=t2, in0=t1, scalar1=1.0 / 3.0,
                                    scalar2=1.0, op0=OP.mult, op1=OP.add)
            nc.vector.tensor_mul(out=t3, in0=t1, in1=t2)
            nc.vector.tensor_scalar(out=t2, in0=t3, scalar1=0.5,
                                    scalar2=1.0, op0=OP.mult, op1=OP.add)
            nc.vector.tensor_mul(out=t3, in0=t1, in1=t2)                 # expm1
            nc.scalar.activation(out=t2, in_=tspf, func=AF.Exp, scale=0.2)
            nc.vector.tensor_mul(out=t3, in0=t3, in1=t2)
            nc.scalar.activation(out=cm1, in_=t3, func=AF.Sqrt, scale=5.0)
            # nb = 1/(sqrt(ts)+1e-8)
            nc.scalar.activation(out=t0, in_=tsf, func=AF.Sqrt)
            nc.vector.tensor_scalar_add(out=t0, in0=t0, scalar1=1e-8)
            nc.vector.reciprocal(out=cn0, in_=t0)
            # f2 = exp(-.1 ts) / (sqrt(5*(1-exp(-.2 ts))) + 1e-8)
            #   1-exp(-y), y = .2 ts: Taylor (deg 6) below y=0.5 else direct
            nc.vector.tensor_scalar_mul(out=t0, in0=tsf, scalar1=0.2)    # y
            nc.scalar.activation(out=t1, in_=tsf, func=AF.Exp, scale=-0.2)
            nc.vector.tensor_scalar(out=t1, in0=t1, scalar1=-1.0,
                                    scalar2=1.0, op0=OP.mult, op1=OP.add)
            nc.vector.tensor_scalar(out=t2, in0=t0, scalar1=-1.0 / 6.0,
                                    scalar2=1.0, op0=OP.mult, op1=OP.add)
            for k in (5, 4, 3, 2):
                nc.vector.tensor_mul(out=t3, in0=t0, in1=t2)
                nc.vector.tensor_scalar(out=t2, in0=t3, scalar1=-1.0 / k,
                                        scalar2=1.0, op0=OP.mult, op1=OP.add)
            nc.vector.tensor_mul(out=t3, in0=t0, in1=t2)                 # taylor
            nc.vector.tensor_scalar(out=t4, in0=t0, scalar1=0.5, scalar2=None,
                                    op0=OP.is_lt)
            nc.vector.tensor_tensor(out=t3, in0=t3, in1=t1, op=OP.subtract)
            nc.vector.tensor_mul(out=t3, in0=t4, in1=t3)
            nc.vector.tensor_tensor(out=t3, in0=t3, in1=t1, op=OP.add)   # w2
            nc.scalar.activation(out=t3, in_=t3, func=AF.Sqrt, scale=5.0)
            nc.vector.tensor_scalar_add(out=t3, in0=t3, scalar1=1e-8)
            nc.vector.reciprocal(out=t3, in_=t3)
            nc.scalar.activation(out=t0, in_=tsf, func=AF.Exp, scale=-0.1)
            nc.vector.tensor_mul(out=cn1, in0=t0, in1=t3)

            # ---------------- main scan ----------------
            # float32r operands run the matmul at 1 cycle/row instead of
            # fp32's 4 (output free size 512 >= 256). The BIR verifier
            # requires every producer feeding an fp32r matmul to round its
            # output to fp32r, so r/ctmp tiles are allocated as f32r and the
            # stationaries are rounded copies.
            F32R = mybir.dt.float32r
            ctmp = [consts.tile([32, FREE], F32R, tag=f"ctmp{g}", name=f"ctmp{g}")
                    for g in range(G)]
            LR = consts.tile([P, P], F32R)
            nc.vector.tensor_copy(out=LR, in_=L)
            E31R = consts.tile([32, P], F32R)
            nc.vector.tensor_copy(out=E31R, in_=e31)

            def emit_reps():
              for _rep in range(reps):
                for i2 in range(NC2):
                    tsl = slice(i2 * 2 * P, (i2 + 1) * 2 * P)
                    # one 2 MiB load per 256-step chunk (all 32 samples) on
                    # the SP HWDGE ring; stores go on the ACT ring below.
                    nt = nzp.tile([P, NB, L2 * DIM], F32, tag="nz")
                    nc.sync.dma_start(
                        out=nt,
                        in_=nz_in[:, tsl, :].rearrange(
                            "s (p l) d -> p s (l d)", l=L2))
                    o = op_.tile([P, NB, L2 * DIM], F32, tag="o")
                    for g in range(G):
                        sgs = slice(g * S, (g + 1) * S)
                        r = rp.tile([P, S, L2, DIM], F32R, tag="r")
                        nc.vector.tensor_mul(
                            out=r[:, :, :, :].rearrange(
                                "p s l (h d) -> p s l h d", h=H),
                            in0=nt[:, sgs, :].rearrange(
                                "p s (l h d) -> p s l h d", l=L2, h=H),
                            in1=cmul[:, g, :, i2, :, :].to_broadcast(
                                [P, S, L2, H, DH]))
                        ps = psp.tile([P, FREE], F32, tag="ps")
                        re = r[:, :, 0, :]
                        ro = r[:, :, 1, :]
                        nc.tensor.matmul(ps, lhsT=LR, rhs=re,
                                         start=True, stop=False)
                        nc.tensor.matmul(ps, lhsT=LR, rhs=ro,
                                         start=False, stop=(i2 == 0))
                        if i2 > 0:
                            nc.tensor.matmul(ps, lhsT=E31R, rhs=ctmp[g],
                                             start=False, stop=True)
                        if i2 < NC2 - 1:
                            nc.scalar.activation(out=ctmp[g], in_=ps[96:128, :],
                                                 func=AF.Copy)
                        # odd time positions: S_odd * cnorm
                        nc.vector.tensor_mul(
                            out=o[:, sgs, DIM : L2 * DIM].rearrange(
                                "p s (h d) -> p s h d", h=H),
                            in0=ps[:, :].rearrange(
                                "p (s h d) -> p s h d", s=S, h=H),
                            in1=cnorm[:, g, :, i2, 1, :].to_broadcast(
                                [P, S, H, DH]))
                        # even time positions: (S_odd - r_odd) * cnorm
                        te = tep.tile([P, S, DIM], F32, tag="te")
                        nc.vector.tensor_tensor(
                            out=te,
                            in0=ps[:, :].rearrange("p (s d) -> p s d", s=S),
                            in1=ro, op=OP.subtract)
                        nc.gpsimd.tensor_mul(
                            out=o[:, sgs, 0:DIM].rearrange(
                                "p s (h d) -> p s h d", h=H),
                            in0=te[:, :, :].rearrange(
                                "p s (h d) -> p s h d", h=H),
                            in1=cnorm[:, g, :, i2, 0, :].to_broadcast(
                                [P, S, H, DH]))
                    nc.scalar.dma_start(
                        out=out[:, tsl, :].rearrange(
                            "s (p l) d -> p s (l d)", l=L2),
                        in_=o)

            if hw_loop:
                with tc.For_i(0, hw_loop):
                    emit_reps()
            else:
                emit_reps()
    _split_waits(nc)
    return nc


_NC = None


def _get_nc():
    global _NC
    if _NC is None:
        _NC = _build()
    return _NC


def kernel(ts: np.ndarray, noise: np.ndarray) -> np.ndarray:
    ts = np.ascontiguousarray(ts, dtype=np.float32)
    noise = np.ascontiguousarray(noise, dtype=np.float32)
    in_maps = [
        {"ts": ts[c * NB : (c + 1) * NB], "noise": noise[c * NB : (c + 1) * NB]}
        for c in range(N_CORES)
    ]
    res = run_bass_kernel_spmd(_get_nc(), in_maps, core_ids=list(range(N_CORES)))
    return np.concatenate([r["out"] for r in res.results], axis=0)



# revision 43
# speedup vs baseline: 1.9752x; 1.4134x over previous
"""Brownian/OU bridge sampler kernel for Trainium2 (8 NeuronCores).

Problem (per batch element b, time series of length T, DIM=64 channels):
  first 32 channels:  bm = cumsum_t(sqrt(dt)*noise) / (sqrt(t)+1e-8)
  last 32 channels:   ou = e^{-theta t} * cumsum_t(sqrt(e^{2 theta t}-e^{2 theta t'})
                           * sigma/sqrt(2 theta) * noise)
                           / (sigma*sqrt((1-e^{-2 theta t})/(2 theta))+1e-8)

Strategy: pure data parallel over batch (32 samples per core); no cross-core
communication. The rel-err gate (2e-2) leaves a large precision budget, so
noise and the output cross HBM as bf16 (host casts in kernel()), halving the
33.5 MB/core fp32 traffic to 16.8 MB -> ~47 us DMA roofline per core.

Layout: time-QUAD — partition p of a 512-step chunk holds t = 4p+l, l<4, so
every DMA descriptor covers 4 steps x 64 ch x 2 B = 512 B (the SDMA line-rate
knee; smaller descriptors run at half rate). The 512-step cumsum runs as one
bf16 matmul per chunk against an inclusive triangular-ones stationary on the
quad SUMS (rsum): PSUM row q = carry + sum of quads <= q. Within-quad
positions are reconstructed with suffix sums: out_l = (S - suf_l) * n_l.
Cross-chunk carries ride a bf16 matmul selecting PSUM row 127 from an
aligned 32-row stash (ACT copy), exactly as in the fp32 pair-layout variant.

Per-timestep coefficients are precomputed once on a compact [128, (s,i2,l,h)]
layout (flat full-speed ts load + 4 strided PE-transposes), then EXPANDED
along the channel dim into [P][i2][g][s][l][h][d] bf16 tables. The expansion
is rep-invariant preamble work; it makes every steady-state elementwise op
fully packed bf16, which the DVE runs in its 2x 16-bit mode. All elementwise
work lives on the DVE (HW showed Q7/Pool bf16 tensor ops run ~2x slower than
the cost model, so any Pool op straggles the per-half store); the four
norm-muls issue as ONE instruction over an l-major scratch tile. ACT does
PSUM->SBUF bf16 copies, carry stashes, and the store-DMA ring; loads ride SP.

Numerics: exp(2θt)-exp(2θt') is restructured as exp(2θt')*expm1(2θ dt)
(cubic Taylor expm1; dt<=1e-2) and 1-exp(-2θt) uses a degree-6 Taylor/direct
blend at 2θt=0.5 — the coefficient pipeline stays fp32 until the final bf16
rounding. End-to-end error vs the fp32 reference is ~4e-3 (bf16 quantization
of noise/coefficients/output), comfortably inside the 2e-2 gate.
"""
import numpy as np
import ml_dtypes

import bass_rust
import concourse.bass as bass
import concourse.tile as tile
from concourse import mybir
from concourse.bass_utils import run_bass_kernel_spmd

B, T, DIM = 256, 2048, 64
THETA = 0.1
N_CORES = 8
NB = B // N_CORES      # 32 samples per core
P = 128                # partitions
L4 = 4                 # quad: timesteps per partition per chunk
NC4 = T // (L4 * P)    # 4 time chunks of 512 steps
S = 8                  # samples packed per matmul free dim
G = NB // S            # 4 carry chains per core
HS = 16                # samples per DMA (half chunk)
H = 2                  # halves (bm / ou)
DH = DIM // H          # 32
FREE = S * DIM         # 512 = one PSUM bank of fp32

F32 = mybir.dt.float32
F32R = mybir.dt.float32r
BF16 = mybir.dt.bfloat16
AF = mybir.ActivationFunctionType
OP = mybir.AluOpType


def _split_waits(nc, max_waits=1):
    """walrus in this container rejects >1 sem wait per instruction; hoist
    extras onto same-engine NoOps inserted just before the offender."""
    n = 0
    for f in nc.m.functions:
        for blk in f.blocks:
            insts = blk.instructions
            i = 0
            while i < len(insts):
                inst = insts[i]
                si = inst.sync_info
                if si is not None and len(si.on_wait) > max_waits:
                    waits = list(si.on_wait)
                    keep, rest = waits[:max_waits], waits[max_waits:]
                    nops = []
                    for j in range(0, len(rest), max_waits):
                        nop = bass_rust.InstNoOp(name=f"I-ws-{n}", ins=[], outs=[])
                        n += 1
                        nop.engine = inst.engine
                        nop.sync_info = mybir.SyncInfo(
                            on_wait=rest[j : j + max_waits], on_update=[])
                        nops.append(nop)
                    inst.sync_info = mybir.SyncInfo(
                        on_wait=keep, on_update=list(si.on_update))
                    for k, nop in enumerate(nops):
                        insts.insert(i + k, nop)
                    i += len(nops)
                i += 1
    return nc


def _strided(ap_full, offset_elems, step, count):
    """[P, count] view of a tile's free space at element offset with stride."""
    return bass.AP(
        tensor=ap_full.tensor,
        offset=ap_full.offset + offset_elems,
        ap=[list(ap_full.ap[0]), [step, count]],
    )


def _build(reps: int = 1, hw_loop: int = 0):
    nc = bass.Bass("TRN2")
    ts_in = nc.dram_tensor("ts", [NB, T, 1], F32, kind="ExternalInput")
    nz_in = nc.dram_tensor("noise", [NB, T, DIM], BF16, kind="ExternalInput")
    out = nc.dram_tensor("out", [NB, T, DIM], BF16, kind="ExternalOutput")

    ts_flat = ts_in[:, :, 0].rearrange("s t -> (s t)")

    with tile.TileContext(nc) as tc:
        with (
            tc.tile_pool(name="consts", bufs=1) as consts,
            tc.tile_pool(name="nzp", bufs=3) as nzp,
            tc.tile_pool(name="op_", bufs=2) as op_,
            tc.tile_pool(name="psp", bufs=4, space="PSUM") as psp,
        ):
            # ---------------- constants ----------------
            cw0_cm = tc.tile_pool(name="cw0", bufs=1)
            cw0 = cw0_cm.__enter__()
            ones_t = cw0.tile([P, P], F32)
            nc.vector.memset(ones_t, 1.0)
            L = cw0.tile([P, P], F32)          # L[u, q] = 1 if u <= q
            nc.gpsimd.affine_select(
                out=L, in_=ones_t, pattern=[[1, P]], compare_op=OP.is_ge,
                fill=0.0, base=0, channel_multiplier=-1)
            LB = consts.tile([P, P], BF16)        # bf16 stationary
            nc.vector.tensor_copy(out=LB, in_=L)
            e31 = cw0.tile([32, P], F32)       # row 31 ones, else 0
            nc.gpsimd.affine_select(
                out=e31, in_=ones_t[0:32, :], pattern=[[0, P]],
                compare_op=OP.is_equal, fill=0.0, base=-31,
                channel_multiplier=1)
            E31R = consts.tile([32, P], BF16)
            nc.vector.tensor_copy(out=E31R, in_=e31)
            ident = cw0.tile([P, P], F32)      # identity for PE transpose
            nc.gpsimd.affine_select(
                out=ident, in_=ones_t, pattern=[[-1, P]],
                compare_op=OP.is_equal, fill=0.0, base=0,
                channel_multiplier=1)

            # expanded per-timestep coefficient tables (rep-invariant):
            # [P][i2][g][s][l][h][d], innermost d packed so every steady-state
            # elementwise op qualifies for the DVE 16-bit fast path.
            cmulx = consts.tile([P, NC4, G, S, L4, H, DH], BF16)
            cnormx = consts.tile([P, NC4, G, S, L4, H, DH], BF16)
            ctmp = [consts.tile([32, FREE], BF16, tag=f"ctmp{g}", name=f"ctmp{g}")
                    for g in range(G)]

            # -------- compact coefficients: [P, (s, i2, l, h)] --------
            # flat time index f = s*2048 + 512*i2 + 4p + l = q*512 + m with
            # q = 4s + i2 (partition of the flat load), m = 4p + l; the four
            # strided PE-transposes T_l[p, q] land at n = 4q + l.
            with (
                tc.tile_pool(name="cw", bufs=1) as cw,
                tc.tile_pool(name="trps", bufs=1, space="PSUM") as trps,
                tc.tile_pool(name="cwp", bufs=1, space="PSUM") as cwp,
                nc.allow_low_precision(
                    reason="coefficients are rounded to bf16 by design; "
                           "all math stays fp32 until the final store"),
            ):
                s1 = cw.tile([P, 512], F32)       # flat[q*512 + m]
                nc.sync.dma_start(
                    out=s1, in_=ts_flat.rearrange("(p f) -> p f", p=P))
                s1p = cw.tile([P, 512], F32)      # flat[q*512 + m - 1]
                nc.sync.dma_start(
                    out=s1p[1:P, :],
                    in_=bass.AP(tensor=ts_flat.tensor,
                                offset=ts_flat.offset + 511,
                                ap=[[512, P - 1], [1, 512]]))
                nc.sync.dma_start(
                    out=s1p[0:1, 1:512],
                    in_=bass.AP(tensor=ts_flat.tensor, offset=ts_flat.offset,
                                ap=[[0, 1], [1, 511]]))
                nc.vector.memset(s1p[0:1, 0:1], 0.0)

                ts_c = cw.tile([P, 32, NC4, L4], F32)   # (s, i2, l)
                tsp_c = cw.tile([P, 32, NC4, L4], F32)
                tsf = ts_c[:, :, :, :].rearrange("p s i l -> p (s i l)")
                tspf = tsp_c[:, :, :, :].rearrange("p s i l -> p (s i l)")
                for src, dstf in ((s1, tsf), (s1p, tspf)):
                    for lv in range(L4):
                        pst = trps.tile([P, P], F32, tag="trp",
                                        name=f"trp{lv}")
                        nc.tensor.transpose(
                            out=pst,
                            in_=_strided(src[:, :], lv, L4, P),
                            identity=ident)
                        nc.vector.tensor_copy(
                            out=_strided(dstf, lv, L4, P), in_=pst)
                # each sample's t=0 has predecessor time 0
                nc.vector.memset(tsp_c[:1, :, 0:1, 0:1], 0.0)

                cmul = cw.tile([P, 32, NC4, L4, H], BF16)
                cnorm = cw.tile([P, 32, NC4, L4, H], BF16)
                cmf = cmul[:, :, :, :, :].rearrange("p s i l h -> p (s i l h)")
                cnf = cnorm[:, :, :, :, :].rearrange("p s i l h -> p (s i l h)")
                NF = 32 * NC4 * L4  # 512
                cm0 = _strided(cmf, 0, 2, NF)
                cm1 = _strided(cmf, 1, 2, NF)
                cn0 = _strided(cnf, 0, 2, NF)
                cn1 = _strided(cnf, 1, 2, NF)

                t0 = cw.tile([P, NF], F32, tag="t0")
                t1 = cw.tile([P, NF], F32, tag="t1")
                t2 = cwp.tile([P, NF], F32, tag="t2")
                t3 = s1       # transpose sources are dead here; reuse
                t4 = s1p

                # db = sqrt(ts - tsp)  (fp32 subtraction is exact here)
                nc.vector.tensor_tensor(out=t0, in0=tsf, in1=tspf,
                                        op=OP.subtract)
                nc.scalar.activation(out=cm0, in_=t0, func=AF.Sqrt)
                # dou = sqrt(5 * exp(.2 tsp) * expm1(.2 (ts-tsp)))
                nc.vector.tensor_scalar_mul(out=t1, in0=t0, scalar1=0.2)
                nc.vector.tensor_scalar(out=t2, in0=t1, scalar1=1.0 / 3.0,
                                        scalar2=1.0, op0=OP.mult, op1=OP.add)
                nc.vector.tensor_mul(out=t3, in0=t1, in1=t2)
                nc.vector.tensor_scalar(out=t2, in0=t3, scalar1=0.5,
                                        scalar2=1.0, op0=OP.mult, op1=OP.add)
                nc.vector.tensor_mul(out=t3, in0=t1, in1=t2)      # expm1
                nc.scalar.activation(out=t2, in_=tspf, func=AF.Exp, scale=0.2)
                nc.vector.tensor_mul(out=t3, in0=t3, in1=t2)
                nc.scalar.activation(out=cm1, in_=t3, func=AF.Sqrt, scale=5.0)
                # nb = 1/(sqrt(ts)+1e-8)
                nc.scalar.activation(out=t0, in_=tsf, func=AF.Sqrt)
                nc.vector.tensor_scalar_add(out=t0, in0=t0, scalar1=1e-8)
                nc.vector.reciprocal(out=cn0, in_=t0)
                # f2 = exp(-.1 ts) / (sqrt(5*(1-exp(-.2 ts))) + 1e-8)
                #   1-exp(-y), y = .2 ts: Taylor (deg 6) below y=0.5 else direct
                nc.vector.tensor_scalar_mul(out=t0, in0=tsf, scalar1=0.2)
                nc.scalar.activation(out=t1, in_=tsf, func=AF.Exp, scale=-0.2)
                nc.vector.tensor_scalar(out=t1, in0=t1, scalar1=-1.0,
                                        scalar2=1.0, op0=OP.mult, op1=OP.add)
                nc.vector.tensor_scalar(out=t2, in0=t0, scalar1=-1.0 / 6.0,
                                        scalar2=1.0, op0=OP.mult, op1=OP.add)
                for k in (5, 4, 3, 2):
                    nc.vector.tensor_mul(out=t3, in0=t0, in1=t2)
                    nc.vector.tensor_scalar(out=t2, in0=t3, scalar1=-1.0 / k,
                                            scalar2=1.0, op0=OP.mult,
                                            op1=OP.add)
                nc.vector.tensor_mul(out=t3, in0=t0, in1=t2)      # taylor
                nc.vector.tensor_scalar(out=t4, in0=t0, scalar1=0.5,
                                        scalar2=None, op0=OP.is_lt)
                nc.vector.tensor_tensor(out=t3, in0=t3, in1=t1,
                                        op=OP.subtract)
                nc.vector.tensor_mul(out=t3, in0=t4, in1=t3)
                nc.vector.tensor_tensor(out=t3, in0=t3, in1=t1, op=OP.add)
                nc.scalar.activation(out=t3, in_=t3, func=AF.Sqrt, scale=5.0)
                nc.vector.tensor_scalar_add(out=t3, in0=t3, scalar1=1e-8)
                nc.vector.reciprocal(out=t3, in_=t3)
                nc.scalar.activation(out=t0, in_=tsf, func=AF.Exp, scale=-0.1)
                nc.vector.tensor_mul(out=cn1, in0=t0, in1=t3)

                # -------- expand along d into the packed bf16 tables --------
                for comp, xt in ((cmul, cmulx), (cnorm, cnormx)):
                    src = comp[:, :, :, :, :].rearrange(
                        "p (g s) i l h -> p i g s l h", g=G)
                    nc.vector.tensor_copy(
                        out=xt[:, :, :, :, :, :, :],
                        in_=src.to_broadcast([P, NC4, G, S, L4, H, DH]))

            cw0_cm.__exit__(None, None, None)
            # scratch opens after the preamble workspace frees its SBUF, so
            # its 3 generations fit without raising the preamble peak.
            scr_cm = tc.tile_pool(name="scr", bufs=3)
            scr = scr_cm.__enter__()
            # ---------------- main scan ----------------
            # Software-pipelined with a one-half skew: the DVE is in-order,
            # so half k's subs (which wait on the ACT PSUM->SBUF copy) are
            # emitted AFTER half k+1's independent r-mul/tree — the DVE never
            # idles on the matmul->ACT->sem latency.
            def emit_front(i2, hf):
                tsl = slice(i2 * L4 * P, (i2 + 1) * L4 * P)
                sh = slice(hf * HS, (hf + 1) * HS)
                nt = nzp.tile([P, HS, L4, DIM], BF16, tag="nz")
                nc.sync.dma_start(
                    out=nt,
                    in_=nz_in[sh, tsl, :].rearrange(
                        "s (p l) d -> p s l d", l=L4))
                gp = slice(hf * (HS // S), hf * (HS // S) + HS // S)
                # r = noise * cmul, in place over the load tile; elementwise
                # runs at 16-sample granularity, matmuls per 8-sample bank.
                ntv = nt[:, :, :, :].rearrange(
                    "p (g s) l (h d) -> p g s l h d", g=HS // S, h=H)
                nc.vector.tensor_mul(
                    out=ntv, in0=ntv, in1=cmulx[:, i2, gp, :, :, :, :])
                # suffix sums within the quad; one l-major scratch tile:
                # slot 0 = suf0/a0, 1 = suf1/a1, 2 = rsum/a2, 3 = sb, so the
                # four norm-muls later collapse into a single instruction.
                sct = scr.tile([P, L4, HS, DIM], BF16, tag="sct")
                nc.vector.tensor_tensor(
                    out=sct[:, 1, :, :], in0=nt[:, :, 3, :],
                    in1=nt[:, :, 2, :], op=OP.add)
                nc.vector.tensor_tensor(
                    out=sct[:, 0, :, :], in0=sct[:, 1, :, :],
                    in1=nt[:, :, 1, :], op=OP.add)
                nc.vector.tensor_tensor(
                    out=sct[:, 2, :, :], in0=sct[:, 0, :, :],
                    in1=nt[:, :, 0, :], op=OP.add)
                # inclusive cumsum over quads + carry, per 8 samples
                for g2 in range(HS // S):
                    g = hf * (HS // S) + g2
                    sgs = slice(g2 * S, (g2 + 1) * S)
                    ps = psp.tile([P, FREE], F32, tag="ps")
                    nc.tensor.matmul(
                        ps, lhsT=LB, rhs=sct[:, 2, sgs, :],
                        start=True, stop=(i2 == 0))
                    if i2 > 0:
                        nc.tensor.matmul(ps, lhsT=E31R, rhs=ctmp[g],
                                         start=False, stop=True)
                    if i2 < NC4 - 1:
                        nc.scalar.activation(
                            out=ctmp[g], in_=ps[96:128, :], func=AF.Copy)
                    nc.scalar.activation(
                        out=sct[:, 3, sgs, :],
                        in_=ps[:, :].rearrange("p (s d) -> p s d", s=S),
                        func=AF.Copy)
                return dict(i2=i2, hf=hf, tsl=tsl, sh=sh, nt=nt, sct=sct,
                            gp=gp)

            def emit_back(st):
                i2, gp, nt, sct = st["i2"], st["gp"], st["nt"], st["sct"]
                o = op_.tile([P, HS, L4, DIM], BF16, tag="o")
                # a_l = S - suf_l in place over the suf slots (suf_3 = 0):
                # slots 0,1 in one instruction with sb l-broadcast (stride 0)
                sb3 = sct[:, 3, :, :]
                nc.vector.tensor_tensor(
                    out=sct[:, 0, :, :], in0=sb3,
                    in1=sct[:, 0, :, :], op=OP.subtract)
                nc.vector.tensor_tensor(
                    out=sct[:, 1, :, :], in0=sb3,
                    in1=sct[:, 1, :, :], op=OP.subtract)
                nc.vector.tensor_tensor(
                    out=sct[:, 2, :, :], in0=sb3,
                    in1=nt[:, :, 3, :], op=OP.subtract)
                # out_l = a_l * n_l for all four l in ONE packed instruction
                nc.vector.tensor_mul(
                    out=o[:, :, :, :].rearrange(
                        "p s l (h d) -> p l s h d", h=H),
                    in0=sct[:, :, :, :].rearrange(
                        "p l s (h d) -> p l s h d", h=H),
                    in1=cnormx[:, i2, gp, :, :, :, :].rearrange(
                        "p g s l h d -> p l (g s) h d"))
                nc.scalar.dma_start(
                    out=out[st["sh"], st["tsl"], :].rearrange(
                        "s (p l) d -> p s l d", l=L4),
                    in_=o)

            def emit_reps():
                pend = None
                for _rep in range(reps):
                    for i2 in range(NC4):
                        for hf in range(NB // HS):
                            st = emit_front(i2, hf)
                            if pend is not None:
                                emit_back(pend)
                            pend = st
                if pend is not None:
                    emit_back(pend)

            if hw_loop:
                with tc.For_i(0, hw_loop):
                    emit_reps()
            else:
                emit_reps()
            scr_cm.__exit__(None, None, None)
    _split_waits(nc)
    return nc


_NC = None


def _get_nc():
    global _NC
    if _NC is None:
        _NC = _build()
    return _NC


def kernel(ts: np.ndarray, noise: np.ndarray) -> np.ndarray:
    ts = np.ascontiguousarray(ts, dtype=np.float32)
    noise_bf = np.ascontiguousarray(noise, dtype=np.float32).astype(
        ml_dtypes.bfloat16)
    in_maps = [
        {"ts": ts[c * NB : (c + 1) * NB],
         "noise": noise_bf[c * NB : (c + 1) * NB]}
        for c in range(N_CORES)
    ]
    res = run_bass_kernel_spmd(_get_nc(), in_maps, core_ids=list(range(N_CORES)))
    return np.concatenate(
        [r["out"].astype(np.float32) for r in res.results], axis=0)


# revision 44
# speedup vs baseline: 2.0914x; 1.0589x over previous
"""Brownian/OU bridge sampler kernel for Trainium2 (8 NeuronCores).

Problem (per batch element b, time series of length T, DIM=64 channels):
  first 32 channels:  bm = cumsum_t(sqrt(dt)*noise) / (sqrt(t)+1e-8)
  last 32 channels:   ou = e^{-theta t} * cumsum_t(sqrt(e^{2 theta t}-e^{2 theta t'})
                           * sigma/sqrt(2 theta) * noise)
                           / (sigma*sqrt((1-e^{-2 theta t})/(2 theta))+1e-8)

Strategy: pure data parallel over batch (32 samples per core); no cross-core
communication. The rel-err gate (2e-2) leaves a large precision budget, so
noise and the output cross HBM as bf16 (host casts in kernel()), halving the
33.5 MB/core fp32 traffic to 16.8 MB -> ~47 us DMA roofline per core.

Layout: time-QUAD — partition p of a 512-step chunk holds t = 4p+l, l<4, so
every DMA descriptor covers 4 steps x 64 ch x 2 B = 512 B (the SDMA line-rate
knee; smaller descriptors run at half rate). The 512-step cumsum runs as one
bf16 matmul per chunk against an inclusive triangular-ones stationary on the
quad SUMS (rsum): PSUM row q = carry + sum of quads <= q. Within-quad
positions are reconstructed with suffix sums: out_l = (S - suf_l) * n_l.
Cross-chunk carries ride a bf16 matmul selecting PSUM row 127 from an
aligned 32-row stash (ACT copy), exactly as in the fp32 pair-layout variant.

Per-timestep coefficients are precomputed once on a compact [128, (s,i2,l,h)]
layout (flat full-speed ts load + 4 strided PE-transposes), then EXPANDED
along the channel dim into [P][i2][g][s][l][h][d] bf16 tables. The expansion
is rep-invariant preamble work; it makes every steady-state elementwise op
fully packed bf16, which the DVE runs in its 2x 16-bit mode. All elementwise
work lives on the DVE (HW showed Q7/Pool bf16 tensor ops run ~2x slower than
the cost model, so any Pool op straggles the per-half store); the four
norm-muls issue as ONE instruction over an l-major scratch tile. ACT does
PSUM->SBUF bf16 copies, carry stashes, and the store-DMA ring; loads ride SP.

Numerics: exp(2θt)-exp(2θt') is restructured as exp(2θt')*expm1(2θ dt)
(cubic Taylor expm1; dt<=1e-2) and 1-exp(-2θt) uses a degree-6 Taylor/direct
blend at 2θt=0.5 — the coefficient pipeline stays fp32 until the final bf16
rounding. End-to-end error vs the fp32 reference is ~4e-3 (bf16 quantization
of noise/coefficients/output), comfortably inside the 2e-2 gate.
"""
import numpy as np
import ml_dtypes

import bass_rust
import concourse.bass as bass
import concourse.tile as tile
from concourse import mybir
from concourse.bass_utils import run_bass_kernel_spmd

B, T, DIM = 256, 2048, 64
THETA = 0.1
N_CORES = 8
NB = B // N_CORES      # 32 samples per core
P = 128                # partitions
L4 = 4                 # quad: timesteps per partition per chunk
NC4 = T // (L4 * P)    # 4 time chunks of 512 steps
S = 8                  # samples packed per matmul free dim
G = NB // S            # 4 carry chains per core
HS = 16                # samples per DMA (half chunk)
H = 2                  # halves (bm / ou)
DH = DIM // H          # 32
FREE = S * DIM         # 512 = one PSUM bank of fp32

F32 = mybir.dt.float32
F32R = mybir.dt.float32r
BF16 = mybir.dt.bfloat16
AF = mybir.ActivationFunctionType
OP = mybir.AluOpType


def _split_waits(nc, max_waits=1):
    """walrus in this container rejects >1 sem wait per instruction; hoist
    extras onto same-engine NoOps inserted just before the offender."""
    n = 0
    for f in nc.m.functions:
        for blk in f.blocks:
            insts = blk.instructions
            i = 0
            while i < len(insts):
                inst = insts[i]
                si = inst.sync_info
                if si is not None and len(si.on_wait) > max_waits:
                    waits = list(si.on_wait)
                    keep, rest = waits[:max_waits], waits[max_waits:]
                    nops = []
                    for j in range(0, len(rest), max_waits):
                        nop = bass_rust.InstNoOp(name=f"I-ws-{n}", ins=[], outs=[])
                        n += 1
                        nop.engine = inst.engine
                        nop.sync_info = mybir.SyncInfo(
                            on_wait=rest[j : j + max_waits], on_update=[])
                        nops.append(nop)
                    inst.sync_info = mybir.SyncInfo(
                        on_wait=keep, on_update=list(si.on_update))
                    for k, nop in enumerate(nops):
                        insts.insert(i + k, nop)
                    i += len(nops)
                i += 1
    return nc


def _strided(ap_full, offset_elems, step, count):
    """[P, count] view of a tile's free space at element offset with stride."""
    return bass.AP(
        tensor=ap_full.tensor,
        offset=ap_full.offset + offset_elems,
        ap=[list(ap_full.ap[0]), [step, count]],
    )


def _build(reps: int = 1, hw_loop: int = 0):
    nc = bass.Bass("TRN2")
    ts_in = nc.dram_tensor("ts", [NB, T, 1], F32, kind="ExternalInput")
    nz_in = nc.dram_tensor("noise", [NB, T, DIM], BF16, kind="ExternalInput")
    out = nc.dram_tensor("out", [NB, T, DIM], BF16, kind="ExternalOutput")

    ts_flat = ts_in[:, :, 0].rearrange("s t -> (s t)")

    with tile.TileContext(nc) as tc:
        with (
            tc.tile_pool(name="consts", bufs=1) as consts,
            tc.tile_pool(name="nzp", bufs=3) as nzp,
            tc.tile_pool(name="op_", bufs=2) as op_,
            tc.tile_pool(name="psp", bufs=4, space="PSUM") as psp,
        ):
            # ---------------- constants ----------------
            cw0_cm = tc.tile_pool(name="cw0", bufs=1)
            cw0 = cw0_cm.__enter__()
            ones_t = cw0.tile([P, P], F32)
            nc.vector.memset(ones_t, 1.0)
            L = cw0.tile([P, P], F32)          # L[u, q] = 1 if u <= q
            nc.gpsimd.affine_select(
                out=L, in_=ones_t, pattern=[[1, P]], compare_op=OP.is_ge,
                fill=0.0, base=0, channel_multiplier=-1)
            LB = consts.tile([P, P], BF16)        # bf16 stationary
            nc.vector.tensor_copy(out=LB, in_=L)
            e31 = cw0.tile([32, P], F32)       # row 31 ones, else 0
            nc.gpsimd.affine_select(
                out=e31, in_=ones_t[0:32, :], pattern=[[0, P]],
                compare_op=OP.is_equal, fill=0.0, base=-31,
                channel_multiplier=1)
            E31R = consts.tile([32, P], BF16)
            nc.vector.tensor_copy(out=E31R, in_=e31)
            ident = cw0.tile([P, P], F32)      # identity for PE transpose
            nc.gpsimd.affine_select(
                out=ident, in_=ones_t, pattern=[[-1, P]],
                compare_op=OP.is_equal, fill=0.0, base=0,
                channel_multiplier=1)

            # expanded per-timestep coefficient tables (rep-invariant):
            # [P][i2][g][s][l][h][d], innermost d packed so every steady-state
            # elementwise op qualifies for the DVE 16-bit fast path.
            cmulx = consts.tile([P, NC4, G, S, L4, H, DH], BF16)
            cnormx = consts.tile([P, NC4, G, S, L4, H, DH], BF16)
            ctmp = [consts.tile([32, FREE], BF16, tag=f"ctmp{g}", name=f"ctmp{g}")
                    for g in range(G)]

            # -------- compact coefficients: [P, (s, i2, l, h)] --------
            # flat time index f = s*2048 + 512*i2 + 4p + l = q*512 + m with
            # q = 4s + i2 (partition of the flat load), m = 4p + l; the four
            # strided PE-transposes T_l[p, q] land at n = 4q + l.
            with (
                tc.tile_pool(name="cw", bufs=1) as cw,
                tc.tile_pool(name="trps", bufs=1, space="PSUM") as trps,
                tc.tile_pool(name="cwp", bufs=1, space="PSUM") as cwp,
                nc.allow_low_precision(
                    reason="coefficients are rounded to bf16 by design; "
                           "all math stays fp32 until the final store"),
            ):
                s1 = cw.tile([P, 512], F32)       # flat[q*512 + m]
                nc.sync.dma_start(
                    out=s1, in_=ts_flat.rearrange("(p f) -> p f", p=P))
                s1p = cw.tile([P, 512], F32)      # flat[q*512 + m - 1]
                nc.sync.dma_start(
                    out=s1p[1:P, :],
                    in_=bass.AP(tensor=ts_flat.tensor,
                                offset=ts_flat.offset + 511,
                                ap=[[512, P - 1], [1, 512]]))
                nc.sync.dma_start(
                    out=s1p[0:1, 1:512],
                    in_=bass.AP(tensor=ts_flat.tensor, offset=ts_flat.offset,
                                ap=[[0, 1], [1, 511]]))
                nc.vector.memset(s1p[0:1, 0:1], 0.0)

                ts_c = cw.tile([P, 32, NC4, L4], F32)   # (s, i2, l)
                tsp_c = cw.tile([P, 32, NC4, L4], F32)
                tsf = ts_c[:, :, :, :].rearrange("p s i l -> p (s i l)")
                tspf = tsp_c[:, :, :, :].rearrange("p s i l -> p (s i l)")
                for src, dstf in ((s1, tsf), (s1p, tspf)):
                    for lv in range(L4):
                        pst = trps.tile([P, P], F32, tag="trp",
                                        name=f"trp{lv}")
                        nc.tensor.transpose(
                            out=pst,
                            in_=_strided(src[:, :], lv, L4, P),
                            identity=ident)
                        nc.vector.tensor_copy(
                            out=_strided(dstf, lv, L4, P), in_=pst)
                # each sample's t=0 has predecessor time 0
                nc.vector.memset(tsp_c[:1, :, 0:1, 0:1], 0.0)

                cmul = cw.tile([P, 32, NC4, L4, H], BF16)
                cnorm = cw.tile([P, 32, NC4, L4, H], BF16)
                cmf = cmul[:, :, :, :, :].rearrange("p s i l h -> p (s i l h)")
                cnf = cnorm[:, :, :, :, :].rearrange("p s i l h -> p (s i l h)")
                NF = 32 * NC4 * L4  # 512
                cm0 = _strided(cmf, 0, 2, NF)
                cm1 = _strided(cmf, 1, 2, NF)
                cn0 = _strided(cnf, 0, 2, NF)
                cn1 = _strided(cnf, 1, 2, NF)

                t0 = cw.tile([P, NF], F32, tag="t0")
                t1 = cw.tile([P, NF], F32, tag="t1")
                t2 = cwp.tile([P, NF], F32, tag="t2")
                t3 = s1       # transpose sources are dead here; reuse
                t4 = s1p

                # db = sqrt(ts - tsp)  (fp32 subtraction is exact here)
                nc.vector.tensor_tensor(out=t0, in0=tsf, in1=tspf,
                                        op=OP.subtract)
                nc.scalar.activation(out=cm0, in_=t0, func=AF.Sqrt)
                # dou = sqrt(5 * exp(.2 tsp) * expm1(.2 (ts-tsp)))
                nc.vector.tensor_scalar_mul(out=t1, in0=t0, scalar1=0.2)
                nc.vector.tensor_scalar(out=t2, in0=t1, scalar1=1.0 / 3.0,
                                        scalar2=1.0, op0=OP.mult, op1=OP.add)
                nc.vector.tensor_mul(out=t3, in0=t1, in1=t2)
                nc.vector.tensor_scalar(out=t2, in0=t3, scalar1=0.5,
                                        scalar2=1.0, op0=OP.mult, op1=OP.add)
                nc.vector.tensor_mul(out=t3, in0=t1, in1=t2)      # expm1
                nc.scalar.activation(out=t2, in_=tspf, func=AF.Exp, scale=0.2)
                nc.vector.tensor_mul(out=t3, in0=t3, in1=t2)
                nc.scalar.activation(out=cm1, in_=t3, func=AF.Sqrt, scale=5.0)
                # nb = 1/(sqrt(ts)+1e-8)
                nc.scalar.activation(out=t0, in_=tsf, func=AF.Sqrt)
                nc.vector.tensor_scalar_add(out=t0, in0=t0, scalar1=1e-8)
                nc.vector.reciprocal(out=cn0, in_=t0)
                # f2 = exp(-.1 ts) / (sqrt(5*(1-exp(-.2 ts))) + 1e-8)
                #   1-exp(-y), y = .2 ts: Taylor (deg 6) below y=0.5 else direct
                nc.vector.tensor_scalar_mul(out=t0, in0=tsf, scalar1=0.2)
                nc.scalar.activation(out=t1, in_=tsf, func=AF.Exp, scale=-0.2)
                nc.vector.tensor_scalar(out=t1, in0=t1, scalar1=-1.0,
                                        scalar2=1.0, op0=OP.mult, op1=OP.add)
                nc.vector.tensor_scalar(out=t2, in0=t0, scalar1=-1.0 / 6.0,
                                        scalar2=1.0, op0=OP.mult, op1=OP.add)
                for k in (5, 4, 3, 2):
                    nc.vector.tensor_mul(out=t3, in0=t0, in1=t2)
                    nc.vector.tensor_scalar(out=t2, in0=t3, scalar1=-1.0 / k,
                                            scalar2=1.0, op0=OP.mult,
                                            op1=OP.add)
                nc.vector.tensor_mul(out=t3, in0=t0, in1=t2)      # taylor
                nc.vector.tensor_scalar(out=t4, in0=t0, scalar1=0.5,
                                        scalar2=None, op0=OP.is_lt)
                nc.vector.tensor_tensor(out=t3, in0=t3, in1=t1,
                                        op=OP.subtract)
                nc.vector.tensor_mul(out=t3, in0=t4, in1=t3)
                nc.vector.tensor_tensor(out=t3, in0=t3, in1=t1, op=OP.add)
                nc.scalar.activation(out=t3, in_=t3, func=AF.Sqrt, scale=5.0)
                nc.vector.tensor_scalar_add(out=t3, in0=t3, scalar1=1e-8)
                nc.vector.reciprocal(out=t3, in_=t3)
                nc.scalar.activation(out=t0, in_=tsf, func=AF.Exp, scale=-0.1)
                nc.vector.tensor_mul(out=cn1, in0=t0, in1=t3)

                # -------- expand along d into the packed bf16 tables --------
                for comp, xt in ((cmul, cmulx), (cnorm, cnormx)):
                    src = comp[:, :, :, :, :].rearrange(
                        "p (g s) i l h -> p i g s l h", g=G)
                    nc.vector.tensor_copy(
                        out=xt[:, :, :, :, :, :, :],
                        in_=src.to_broadcast([P, NC4, G, S, L4, H, DH]))

            cw0_cm.__exit__(None, None, None)
            # scratch opens after the preamble workspace frees its SBUF, so
            # its 3 generations fit without raising the preamble peak.
            scr_cm = tc.tile_pool(name="scr", bufs=3)
            scr = scr_cm.__enter__()
            # ---------------- main scan ----------------
            # Software-pipelined with a one-half skew: the DVE is in-order,
            # so half k's subs (which wait on the ACT PSUM->SBUF copy) are
            # emitted AFTER half k+1's independent r-mul/tree — the DVE never
            # idles on the matmul->ACT->sem latency.
            def emit_front(i2, hf):
                tsl = slice(i2 * L4 * P, (i2 + 1) * L4 * P)
                sh = slice(hf * HS, (hf + 1) * HS)
                nt = nzp.tile([P, HS, L4, DIM], BF16, tag="nz")
                nc.sync.dma_start(
                    out=nt,
                    in_=nz_in[sh, tsl, :].rearrange(
                        "s (p l) d -> p s l d", l=L4))
                gp = slice(hf * (HS // S), hf * (HS // S) + HS // S)
                # r = noise * cmul: l=0..2 in place over the load tile,
                # l=3 into scratch slot 2. Scratch slots (l-major tile):
                # 0 = suf0/a0, 1 = suf1/a1, 2 = r3/a2, 3 = sb.
                sct = scr.tile([P, L4, HS, DIM], BF16, tag="sct")
                ntv = nt[:, :, 0:3, :].rearrange(
                    "p (g s) l (h d) -> p g s l h d", g=HS // S, h=H)
                nc.vector.tensor_mul(
                    out=ntv, in0=ntv, in1=cmulx[:, i2, gp, :, 0:3, :, :])
                nc.vector.tensor_mul(
                    out=sct[:, 2, :, :].rearrange("p s (h d) -> p s h d",
                                                  h=H),
                    in0=nt[:, :, 3, :].rearrange("p s (h d) -> p s h d",
                                                 h=H),
                    in1=cmulx[:, i2, gp, :, 3, :, :].rearrange(
                        "p g s h d -> p (g s) h d"))
                # suffix sums; the full-quad sum never materializes — the
                # PE accumulates L^T suf0 + L^T r0 in PSUM instead.
                nc.vector.tensor_tensor(
                    out=sct[:, 1, :, :], in0=sct[:, 2, :, :],
                    in1=nt[:, :, 2, :], op=OP.add)
                nc.vector.tensor_tensor(
                    out=sct[:, 0, :, :], in0=sct[:, 1, :, :],
                    in1=nt[:, :, 1, :], op=OP.add)
                # inclusive cumsum over quads + carry, per 8 samples
                for g2 in range(HS // S):
                    g = hf * (HS // S) + g2
                    sgs = slice(g2 * S, (g2 + 1) * S)
                    ps = psp.tile([P, FREE], F32, tag="ps")
                    nc.tensor.matmul(
                        ps, lhsT=LB, rhs=sct[:, 0, sgs, :],
                        start=True, stop=False)
                    nc.tensor.matmul(
                        ps, lhsT=LB, rhs=nt[:, sgs, 0, :],
                        start=False, stop=(i2 == 0))
                    if i2 > 0:
                        nc.tensor.matmul(ps, lhsT=E31R, rhs=ctmp[g],
                                         start=False, stop=True)
                    if i2 < NC4 - 1:
                        nc.scalar.activation(
                            out=ctmp[g], in_=ps[96:128, :], func=AF.Copy)
                    nc.scalar.activation(
                        out=sct[:, 3, sgs, :],
                        in_=ps[:, :].rearrange("p (s d) -> p s d", s=S),
                        func=AF.Copy)
                return dict(i2=i2, hf=hf, tsl=tsl, sh=sh, nt=nt, sct=sct,
                            gp=gp)

            def emit_back(st):
                i2, gp, nt, sct = st["i2"], st["gp"], st["nt"], st["sct"]
                o = op_.tile([P, HS, L4, DIM], BF16, tag="o")
                # a_l = S - suf_l in place over the suf slots (suf_3 = 0):
                # slots 0,1 in one instruction with sb l-broadcast (stride 0)
                sb3 = sct[:, 3, :, :]
                nc.vector.tensor_tensor(
                    out=sct[:, 0, :, :], in0=sb3,
                    in1=sct[:, 0, :, :], op=OP.subtract)
                nc.vector.tensor_tensor(
                    out=sct[:, 1, :, :], in0=sb3,
                    in1=sct[:, 1, :, :], op=OP.subtract)
                nc.vector.tensor_tensor(
                    out=sct[:, 2, :, :], in0=sb3,
                    in1=sct[:, 2, :, :], op=OP.subtract)
                # out_l = a_l * n_l for all four l in ONE packed instruction
                nc.vector.tensor_mul(
                    out=o[:, :, :, :].rearrange(
                        "p s l (h d) -> p l s h d", h=H),
                    in0=sct[:, :, :, :].rearrange(
                        "p l s (h d) -> p l s h d", h=H),
                    in1=cnormx[:, i2, gp, :, :, :, :].rearrange(
                        "p g s l h d -> p l (g s) h d"))
                nc.scalar.dma_start(
                    out=out[st["sh"], st["tsl"], :].rearrange(
                        "s (p l) d -> p s l d", l=L4),
                    in_=o)

            def emit_reps():
                pend = None
                for _rep in range(reps):
                    for i2 in range(NC4):
                        for hf in range(NB // HS):
                            st = emit_front(i2, hf)
                            if pend is not None:
                                emit_back(pend)
                            pend = st
                if pend is not None:
                    emit_back(pend)

            if hw_loop:
                with tc.For_i(0, hw_loop):
                    emit_reps()
            else:
                emit_reps()
            scr_cm.__exit__(None, None, None)
    _split_waits(nc)
    return nc


_NC = None


def _get_nc():
    global _NC
    if _NC is None:
        _NC = _build()
    return _NC


def kernel(ts: np.ndarray, noise: np.ndarray) -> np.ndarray:
    ts = np.ascontiguousarray(ts, dtype=np.float32)
    noise_bf = np.ascontiguousarray(noise, dtype=np.float32).astype(
        ml_dtypes.bfloat16)
    in_maps = [
        {"ts": ts[c * NB : (c + 1) * NB],
         "noise": noise_bf[c * NB : (c + 1) * NB]}
        for c in range(N_CORES)
    ]
    res = run_bass_kernel_spmd(_get_nc(), in_maps, core_ids=list(range(N_CORES)))
    return np.concatenate(
        [r["out"].astype(np.float32) for r in res.results], axis=0)
